# revision 31
# baseline (speedup 1.0000x reference)
"""KGAT-RotatE message-passing kernel for 8 Trainium2 NeuronCores (Bass/Tile).

Self-contained: hardcodes the problem shapes. Strategy (v2):
  - Host packs destination nodes into 128-node blocks (<= T*128 incoming edges
    each) and assigns blocks to cores, so every core fully owns the edge
    softmax + segment sums of its destination nodes. The node embedding table
    is shipped in SLOT order (ent_slot) so all three phases gather rows with
    the same srcslot indices and the dst-block rows load with direct DMA.
  - Per block the kernel indirect-DMA-gathers per-edge src rows, builds
    edge-major (oh) and transposed (ohT / ohtT) one-hot matrices on the DVE
    (iota + is_equal against K=1 broadcast matmuls - no PE transposes), forms
    per-edge dst rows and rotation rows as one-hot matmuls, computes the
    RotatE attention in chunked slab ops, folds the softmax denominator into
    per-edge weights w = exp(att-C) * rinv[dst], and accumulates the weighted
    segment sums via matmuls into PSUM.
  - Layer GEMMs run row-major (PE transpose of x1/x2 then matmul); leaky-relu
    is a single fused DVE op; the row L2 norm uses tensor_tensor_reduce so the
    ACT engine only ever runs Exp/Sqrt/Sin (resident tables, no reloads).
  - Between layers the un-normalized egos are AllGathered so the next layer
    can gather arbitrary source rows.
"""

import numpy as np

# ---------------------------------------------------------------- constants
N_NODES = 100000
E_EDGES = 1_000_000
R_REL = 40
D = 64                      # complex half-dim
PI = 3.1415926235897933     # matches the reference
REL_RANGE = (12.0 + 2.0) / D
PHASE_SCALE = PI / REL_RANGE
C_SHIFT = 50.0              # exp(att - C); att in [20.8, 38.0] for this data
NCORES = 8
BLK = 128

_CACHE = {}


class Cfg:
    def __init__(self, n_nodes, nbc, t):
        self.n_nodes = n_nodes      # number of real nodes
        self.nbc = nbc              # blocks per core
        self.t = t                  # edge tiles (of 128) per block
        self.nslot_core = nbc * BLK
        self.nslot = NCORES * self.nslot_core
        self.epb = t * BLK          # max edges per block
        # tile chunks: groups of <=4 tiles, each chunk >=2 tiles when possible
        ch = []
        r = t
        while r > 0:
            c = min(4, r)
            if r - c == 1 and c == 4:
                c = 3
            ch.append(c)
            r -= c
        self.chunks = ch            # e.g. t=10 -> [4,4,2]


FULL_CFG = Cfg(N_NODES, 102, 10)


# ---------------------------------------------------------------- host prep
def _pack_nodes(deg, cfg):
    """Pack nodes into NCORES*nbc bins; cap 128 nodes & cfg.epb edges/bin.
    LPT: each node (desc degree) goes to the least-edge-loaded bin with room."""
    import heapq
    nbins = NCORES * cfg.nbc
    order = np.argsort(-deg, kind="stable")
    heap = [(0, b) for b in range(nbins)]
    heapq.heapify(heap)
    nodes_in = [[] for _ in range(nbins)]
    esum = [0] * nbins
    for nd in order:
        d = int(deg[nd])
        parked = []
        while heap:
            e, b = heapq.heappop(heap)
            if len(nodes_in[b]) < BLK and e + d <= cfg.epb:
                nodes_in[b].append(int(nd))
                esum[b] = e + d
                if len(nodes_in[b]) < BLK:
                    heapq.heappush(heap, (e + d, b))
                break
            elif len(nodes_in[b]) < BLK:
                parked.append((e, b))
        else:
            raise RuntimeError("bin packing failed: no bin with room")
        for p in parked:
            heapq.heappush(heap, p)
    return [(nodes_in[b], esum[b]) for b in range(nbins)]


def _prep(ent, src, dst, typ, cfg):
    deg = np.bincount(dst, minlength=cfg.n_nodes)
    bins = _pack_nodes(deg, cfg)
    nbins = NCORES * cfg.nbc
    # snake-assign bins to cores by edge count for balance
    esums = np.array([b[1] for b in bins])
    bin_order = np.argsort(-esums, kind="stable")
    core_edges = np.zeros(NCORES, np.int64)
    core_bins = [[] for _ in range(NCORES)]
    for b in bin_order:
        c = int(np.argmin(core_edges + np.array(
            [len(core_bins[i]) * 1e9 if len(core_bins[i]) >= cfg.nbc else 0
             for i in range(NCORES)])))
        core_bins[c].append(b)
        core_edges[c] += esums[b]
    core_of = np.zeros(cfg.n_nodes, np.int32)
    blk_of = np.zeros(cfg.n_nodes, np.int32)
    lane_of = np.zeros(cfg.n_nodes, np.int32)
    # slot-ordered embedding table (zero-filled empty slots)
    ent_slot = np.zeros((cfg.nslot, 128), np.float32)
    for c in range(NCORES):
        for bi, b in enumerate(core_bins[c]):
            nodes = np.asarray(bins[b][0], np.int64)
            n = len(nodes)
            base = c * cfg.nslot_core + bi * BLK
            core_of[nodes] = c; blk_of[nodes] = bi
            lane_of[nodes] = np.arange(n, dtype=np.int32)
            ent_slot[base:base + n] = ent[nodes]
    # group edges by (core, block) of their dst
    ec = core_of[dst]; eb = blk_of[dst]
    key = ec.astype(np.int64) * cfg.nbc + eb
    eorder = np.argsort(key, kind="stable")
    counts = np.bincount(key, minlength=nbins)
    starts = np.concatenate([[0], np.cumsum(counts)])
    keyo = key[eorder]
    flat = keyo * cfg.epb + (np.arange(len(src)) - starts[keyo])
    def padded(vals, fill):
        out = np.full(nbins * cfg.epb, fill, vals.dtype)
        out[flat] = vals[eorder]
        return out.reshape(NCORES, cfg.nbc, cfg.epb)
    srcslot = (core_of[src].astype(np.int64) * cfg.nslot_core
               + blk_of[src] * BLK + lane_of[src]).astype(np.int32)
    p_slot = padded(srcslot, np.int32(0))
    p_lane = padded(lane_of[dst].astype(np.float32), np.float32(-1.0))
    p_typ = padded(typ.astype(np.float32), np.float32(0.0))
    # resident SBUF layout [core, 128(lane p), nbc*T]: edge (b, k*128+p)
    def sbize(a):
        return np.ascontiguousarray(
            a.reshape(NCORES, cfg.nbc, cfg.t, BLK).transpose(0, 3, 1, 2)
            .reshape(NCORES, BLK, cfg.nbc * cfg.t))
    return {
        "srcslot": sbize(p_slot), "dstf": sbize(p_lane),
        # row layouts [core, nbc, 1, epb] (edge slot-major: tile k, lane p)
        "dl_row": np.ascontiguousarray(p_lane[:, :, None, :]),
        "ty_row": np.ascontiguousarray(p_typ[:, :, None, :]),
        "ent_slot": ent_slot,
        "core_of": core_of, "blk_of": blk_of, "lane_of": lane_of,
    }


# ---------------------------------------------------------------- bass build
import os as _os
_PHASES = _os.environ.get("K_PHASES", "ABC")
_STAGE = int(_os.environ.get("K_STAGE", "9"))


def _build(cfg):
    import concourse.bass as bass
    import concourse.mybir as mybir
    import concourse.tile as tile
    from concourse import bacc
    from concourse.bass import IndirectOffsetOnAxis

    f32 = mybir.dt.float32
    f32r = mybir.dt.float32r
    i32 = mybir.dt.int32
    Alu = mybir.AluOpType
    Act = mybir.ActivationFunctionType

    nc = bacc.Bacc("TRN2", target_bir_lowering=False, debug=False,
                   num_devices=NCORES)
    NBC, T = cfg.nbc, cfg.t
    CH = cfg.chunks           # tile chunks, e.g. [4,4,2]
    EPB = cfg.epb

    ent = nc.dram_tensor("ent_slot", [cfg.nslot, 128], f32,
                         kind="ExternalInput").ap()
    ent_loc = nc.dram_tensor("ent_loc", [cfg.nslot_core, 128], f32,
                             kind="ExternalInput").ap()
    rel = nc.dram_tensor("rel", [R_REL, D], f32, kind="ExternalInput").ap()
    wts = {}
    for l, (din, dout) in enumerate([(128, 64), (64, 32), (32, 16)]):
        for nm in ("W1", "W2"):
            wts[f"{nm}_{l}"] = nc.dram_tensor(
                f"{nm}_{l}", [din, dout], f32, kind="ExternalInput").ap()
    srcslot = nc.dram_tensor("srcslot", [BLK, NBC * T], i32,
                             kind="ExternalInput").ap()
    dstf = nc.dram_tensor("dstf", [BLK, NBC * T], f32,
                          kind="ExternalInput").ap()
    dl_row_d = nc.dram_tensor("dl_row", [NBC, 1, EPB], f32,
                              kind="ExternalInput").ap()
    ty_row_d = nc.dram_tensor("ty_row", [NBC, 1, EPB], f32,
                              kind="ExternalInput").ap()
    out = nc.dram_tensor("out", [cfg.nslot_core, 240], f32,
                         kind="ExternalOutput").ap()

    rg = [list(range(NCORES))]

    from contextlib import ExitStack
    with tile.TileContext(nc) as tc, ExitStack() as stk:
        const = stk.enter_context(tc.tile_pool(name="const", bufs=1))
        dram = stk.enter_context(tc.tile_pool(name="dram", bufs=1, space="DRAM"))
        io = stk.enter_context(tc.tile_pool(name="io", bufs=3))
        gat = stk.enter_context(tc.tile_pool(name="gat", bufs=2))
        wk = stk.enter_context(tc.tile_pool(name="wk", bufs=2))
        ps = stk.enter_context(tc.tile_pool(name="ps", bufs=1, space="PSUM"))
        acc = stk.enter_context(tc.tile_pool(name="acc", bufs=1, space="PSUM"))

        eg1sh = dram.tile([cfg.nslot_core, 64], f32)
        eg1full = dram.tile([cfg.nslot, 64], f32, addr_space="Shared")
        eg2sh = dram.tile([cfg.nslot_core, 32], f32)
        eg2full = dram.tile([cfg.nslot, 32], f32, addr_space="Shared")

        # ---- constants / tables
        iota_row = const.tile([BLK, BLK], f32)        # [p, j] = j
        nc.gpsimd.iota(iota_row[:], pattern=[[1, BLK]], base=0,
                       channel_multiplier=0,
                       allow_small_or_imprecise_dtypes=True)
        iota_part = const.tile([BLK, 512], f32)       # [p, x] = p
        nc.gpsimd.iota(iota_part[:], pattern=[[0, 512]], base=0,
                       channel_multiplier=1,
                       allow_small_or_imprecise_dtypes=True)
        ones_col = const.tile([BLK, 1], f32)
        nc.vector.memset(ones_col[:], 1.0)
        ones_row = const.tile([1, BLK], f32)
        nc.vector.memset(ones_row[:], 1.0)
        ones_row_r = const.tile([1, BLK], f32r)
        nc.vector.tensor_copy(out=ones_row_r[:], in_=ones_row[:])
        negC = const.tile([BLK, 1], f32)
        nc.vector.memset(negC[:], -C_SHIFT)
        halfsc = const.tile([BLK, 1], f32)
        nc.vector.memset(halfsc[:], 0.5 * PHASE_SCALE)

        rel_sb = const.tile([R_REL, D], f32)
        nc.sync.dma_start(out=rel_sb[:], in_=rel[:])
        # half-angle trig: s = sin(phase/2) with phase/2 in [-pi/2, pi/2]
        sh = const.tile([R_REL, D], f32)
        nc.scalar.activation(sh[:], rel_sb[:], Act.Sin, scale=halfsc[:R_REL, :1])
        ss = const.tile([R_REL, D], f32)
        nc.vector.tensor_tensor(out=ss[:], in0=sh[:], in1=sh[:], op=Alu.mult)
        cos_tab = const.tile([R_REL, D], f32)
        nc.vector.tensor_scalar(out=cos_tab[:], in0=ss[:], scalar1=-2.0,
                                scalar2=1.0, op0=Alu.mult, op1=Alu.add)
        om = const.tile([R_REL, D], f32)
        nc.vector.tensor_scalar(out=om[:], in0=ss[:], scalar1=-1.0,
                                scalar2=1.0, op0=Alu.mult, op1=Alu.add)
        nc.vector.tensor_scalar(out=om[:], in0=om[:], scalar1=0.0,
                                scalar2=None, op0=Alu.max)
        ch_ = const.tile([R_REL, D], f32)
        nc.scalar.activation(ch_[:], om[:], Act.Sqrt)
        sin_tab = const.tile([R_REL, D], f32)
        nc.vector.scalar_tensor_tensor(out=sin_tab[:], in0=sh[:], scalar=2.0,
                                       in1=ch_[:], op0=Alu.mult, op1=Alu.mult)
        cst_tab = const.tile([R_REL, 2 * D], f32)   # [cos | sin]
        nc.vector.tensor_copy(out=cst_tab[:, :D], in_=cos_tab[:])
        nc.vector.tensor_copy(out=cst_tab[:, D:], in_=sin_tab[:])
        snc_tab = const.tile([R_REL, 2 * D], f32)   # [sin | cos]
        nc.vector.tensor_copy(out=snc_tab[:, :D], in_=sin_tab[:])
        nc.vector.tensor_copy(out=snc_tab[:, D:], in_=cos_tab[:])

        w_sb = {}
        for l, (din, dout) in enumerate([(128, 64), (64, 32), (32, 16)]):
            for nm in ("W1", "W2"):
                t_ = const.tile([din, dout], f32, name=f"{nm}_{l}_sb")
                nc.sync.dma_start(out=t_[:], in_=wts[f"{nm}_{l}"][:])
                w_sb[f"{nm}_{l}"] = t_

        # resident per-core arrays
        idx_all = const.tile([BLK, NBC * T], i32)     # srcslot per edge slot
        nc.sync.dma_start(out=idx_all[:], in_=srcslot[:])
        dl_all = const.tile([BLK, NBC * T], f32)      # dst lane per edge slot
        nc.sync.dma_start(out=dl_all[:], in_=dstf[:])
        w_all = const.tile([BLK, NBC * T], f32)       # per-edge softmax weight
        ego1_sb = const.tile([BLK, NBC * 64], f32)
        ego2_sb = const.tile([BLK, NBC * 32], f32)

        def bcast3(ap2d, n_inner):
            return bass.AP(ap2d.tensor, ap2d.offset,
                           [ap2d.ap[0], ap2d.ap[1], [0, n_inner]])

        def build_oh(dl_blk, name):
            """edge-major one-hot oh[p, t, j] = (j == dl[p, t]) (slab op)."""
            oh = wk.tile([BLK, EPB], f32, name=name, tag="oh")
            it = iota_row[:]
            nc.vector.tensor_tensor(
                out=oh[:].rearrange("p (t j) -> p t j", t=T),
                in0=bass.AP(it.tensor, it.offset, [it.ap[0], [0, T], [1, BLK]]),
                in1=bcast3(dl_blk, BLK), op=Alu.is_equal)
            return oh

        def gemm_norm(x1, x2, l, din, dout, ego_out, egsh, ocol, b):
            """row-major GEMM pair + lrelu + ego + L2-norm + writes."""
            outs = []
            for x, nm in ((x1, "W1"), (x2, "W2")):
                xt_ps = ps.tile([BLK, BLK], f32, name=f"xt{l}{nm}",
                                tag="xt")[:din, :]
                nc.tensor.transpose(out=xt_ps[:], in_=x[:, :din],
                                    identity=ident_sb[:])
                xt_sb = wk.tile([BLK, BLK], f32, name=f"xts{l}{nm}",
                                tag="xts")[:din, :]
                nc.vector.tensor_copy(out=xt_sb[:], in_=xt_ps[:])
                o_ps = ps.tile([BLK, 64], f32, name=f"o{l}{nm}",
                               tag="ops")[:, :dout]
                nc.tensor.matmul(out=o_ps[:], lhsT=xt_sb[:],
                                 rhs=w_sb[f"{nm}_{l}"][:], start=True, stop=True)
                sc = wk.tile([BLK, 64], f32, name=f"sc{l}{nm}",
                             tag="sc")[:, :dout]
                nc.vector.tensor_scalar(out=sc[:], in0=o_ps[:], scalar1=0.01,
                                        scalar2=None, op0=Alu.mult)
                o_sb = wk.tile([BLK, 64], f32, name=f"osb{l}{nm}",
                               tag="osb")[:, :dout]
                nc.vector.tensor_tensor(out=o_sb[:], in0=o_ps[:], in1=sc[:],
                                        op=Alu.max)
                outs.append(o_sb)
            nc.vector.tensor_tensor(out=ego_out, in0=outs[0][:],
                                    in1=outs[1][:], op=Alu.add)
            if egsh is not None:
                nc.sync.dma_start(out=egsh[b * BLK:(b + 1) * BLK, :],
                                  in_=ego_out)
            # L2 norm: square then free-axis reduce (both DVE)
            sq = wk.tile([BLK, 64], f32, name=f"nsq{l}", tag="nsq")[:, :dout]
            nc.vector.tensor_tensor(out=sq[:], in0=ego_out, in1=ego_out,
                                    op=Alu.mult)
            nsq = wk.tile([BLK, 1], f32, name=f"nss{l}", tag="nss")
            nc.vector.tensor_reduce(out=nsq[:], in_=sq[:],
                                    axis=mybir.AxisListType.X, op=Alu.add)
            nr = wk.tile([BLK, 1], f32, name=f"nnr{l}", tag="nnr")
            nc.scalar.activation(nr[:], nsq[:], Act.Sqrt)
            nc.vector.tensor_scalar(out=nr[:], in0=nr[:], scalar1=1e-12,
                                    scalar2=None, op0=Alu.max)
            ni = wk.tile([BLK, 1], f32, name=f"nni{l}", tag="nni")
            nc.vector.reciprocal(ni[:], nr[:])
            on = wk.tile([BLK, 64], f32, name=f"non{l}", tag="non")[:, :dout]
            nc.vector.tensor_scalar(out=on[:], in0=ego_out, scalar1=ni[:, :1],
                                    scalar2=None, op0=Alu.mult)
            nc.sync.dma_start(out=out[b * BLK:(b + 1) * BLK, ocol:ocol + dout],
                              in_=on[:])

        ident_sb = const.tile([BLK, BLK], f32)
        from concourse.masks import make_identity
        make_identity(nc, ident_sb[:])

        # ================= phase A: attention + layer 0 =================
        for b in range(NBC):
            idx_b = idx_all[:, b * T:(b + 1) * T]
            dl_b = dl_all[:, b * T:(b + 1) * T]
            dlr = io.tile([1, EPB], f32, name="dlr", tag="dlr")
            nc.sync.dma_start(out=dlr[:], in_=dl_row_d[b])
            dlr_r = io.tile([1, EPB], f32r, name="dlrr", tag="dlrr")
            nc.vector.tensor_copy(out=dlr_r[:], in_=dlr[:])
            tyr = io.tile([1, EPB], f32, name="tyr", tag="tyr")
            nc.sync.dma_start(out=tyr[:], in_=ty_row_d[b])
            tyr_r = io.tile([1, EPB], f32r, name="tyrr", tag="tyrr")
            nc.vector.tensor_copy(out=tyr_r[:], in_=tyr[:])

            h_slab = gat.tile([BLK, T * 128], f32, name="h_slab", tag="h_slab")
            for k in range(T):
                nc.gpsimd.indirect_dma_start(
                    out=h_slab[:, k * 128:(k + 1) * 128], out_offset=None,
                    in_=ent[:],
                    in_offset=IndirectOffsetOnAxis(ap=idx_b[:, k:k + 1], axis=0))
            eblk = gat.tile([BLK, 128], f32, name="eblk", tag="eblk")
            nc.sync.dma_start(out=eblk[:], in_=ent_loc[b * BLK:(b + 1) * BLK, :])

            nc.sync.dma_start(out=out[b * BLK:(b + 1) * BLK, 0:128],
                              in_=eblk[:])
            if _STAGE < 2:
                continue
            oh = build_oh(dl_b, "ohA")
            # transposed one-hots + per-tile t/rot matmuls + att, chunked
            ohT = wk.tile([BLK, EPB], f32, name="ohT", tag="ohT")
            sq_slab = wk.tile([BLK, T * D], f32, name="sqs", tag="sqs")
            att_blk = wk.tile([BLK, T], f32, name="attb", tag="attb")
            k0 = 0
            for ci, cw in enumerate(CH):
                cols = cw * BLK           # chunk width in edge slots
                c0 = k0 * BLK
                sl = slice(c0, c0 + cols)
                # dlb = broadcast dl_row chunk to 128 partitions (K=1 matmul)
                dlb = ps.tile([BLK, 512], f32, name="dlb", tag="bc")[:, :cols]
                nc.tensor.matmul(out=dlb[:], lhsT=ones_row_r[:],
                                 rhs=dlr_r[:, sl], start=True, stop=True)
                nc.vector.tensor_tensor(out=ohT[:, sl], in0=dlb[:],
                                        in1=iota_part[:, :cols],
                                        op=Alu.is_equal)
                # tyb = broadcast type row chunk to 40 partitions
                tyb = ps.tile([R_REL, 512], f32, name="tyb",
                              tag="bc")[:, :cols]
                nc.tensor.matmul(out=tyb[:], lhsT=ones_row_r[:, :R_REL],
                                 rhs=tyr_r[:, sl], start=True, stop=True)
                ohtT = wk.tile([R_REL, 512], f32, name="ohtT",
                               tag="ohtT")[:, :cols]
                nc.vector.tensor_tensor(out=ohtT[:], in0=tyb[:],
                                        in1=iota_part[:R_REL, :cols],
                                        op=Alu.is_equal)
                if _STAGE < 3:
                    k0 += cw
                    continue
                # per-tile matmuls into chunk PSUM slabs
                t_c = ps.tile([BLK, 512], f32, name="t_c", tag="t_c")[:, :cols]
                rot1 = ps.tile([BLK, 512], f32, name="rot1",
                               tag="rot1")[:, :cols]
                rot2 = ps.tile([BLK, 512], f32, name="rot2",
                               tag="rot2")[:, :cols]
                for kk in range(cw):
                    k = k0 + kk
                    esl = slice((k0 + kk) * BLK, (k0 + kk + 1) * BLK)
                    csl = slice(kk * BLK, (kk + 1) * BLK)
                    nc.tensor.matmul(out=t_c[:, csl], lhsT=ohT[:, esl],
                                     rhs=eblk[:], start=True, stop=True)
                    nc.tensor.matmul(out=rot1[:, csl],
                                     lhsT=ohtT[:, csl][:R_REL, :],
                                     rhs=cst_tab[:], start=True, stop=True)
                    nc.tensor.matmul(out=rot2[:, csl],
                                     lhsT=ohtT[:, csl][:R_REL, :],
                                     rhs=snc_tab[:], start=True, stop=True)
                if _STAGE < 4:
                    k0 += cw
                    continue
                # chunk slab ops: P1/P2, folds, squares
                hsl = h_slab[:, sl]
                P1 = wk.tile([BLK, 512], f32, name="P1", tag="P1")[:, :cols]
                nc.vector.tensor_tensor(out=P1[:], in0=hsl, in1=rot1[:],
                                        op=Alu.mult)
                P2 = wk.tile([BLK, 512], f32, name="P2", tag="P2")[:, :cols]
                nc.vector.tensor_tensor(out=P2[:], in0=hsl, in1=rot2[:],
                                        op=Alu.mult)

                def v3(ap2, lo):
                    # [p, cw*128] AP -> [p, cw, 64] view of re/im halves
                    r = ap2.rearrange("p (c x) -> p c x", c=cw)
                    return r[:, :, lo:lo + D]
                ri = wk.tile([BLK, 512], f32, name="ri", tag="ri")[:, :cols]
                # re_s = P1.re - P1.im - t.re
                nc.vector.tensor_tensor(out=v3(ri[:], 0), in0=v3(P1[:], 0),
                                        in1=v3(P1[:], D), op=Alu.subtract)
                nc.vector.tensor_tensor(out=v3(ri[:], 0), in0=v3(ri[:], 0),
                                        in1=v3(t_c[:], 0), op=Alu.subtract)
                # im_s = P2.re + P2.im - t.im
                nc.vector.tensor_tensor(out=v3(ri[:], D), in0=v3(P2[:], 0),
                                        in1=v3(P2[:], D), op=Alu.add)
                nc.vector.tensor_tensor(out=v3(ri[:], D), in0=v3(ri[:], D),
                                        in1=v3(t_c[:], D), op=Alu.subtract)
                # sq = re_s^2 + im_s^2 -> sq_slab tiles [p, cw, 64]
                sqv = sq_slab[:, k0 * D:(k0 + cw) * D].rearrange(
                    "p (c x) -> p c x", c=cw)
                nc.vector.tensor_tensor(out=v3(ri[:], 0), in0=v3(ri[:], 0),
                                        in1=v3(ri[:], 0), op=Alu.mult)
                nc.vector.tensor_tensor(out=v3(ri[:], D), in0=v3(ri[:], D),
                                        in1=v3(ri[:], D), op=Alu.mult)
                nc.vector.tensor_tensor(out=sqv, in0=v3(ri[:], 0),
                                        in1=v3(ri[:], D), op=Alu.add)
                # att per tile: sum_d sqrt(sq)
                for kk in range(cw):
                    k = k0 + kk
                    mag = wk.tile([BLK, D], f32, name="mag", tag="mag")
                    nc.scalar.activation(
                        mag[:], sq_slab[:, k * D:(k + 1) * D], Act.Sqrt,
                        accum_out=att_blk[:, k:k + 1])
                k0 += cw
            if _STAGE < 5:
                continue
            # ecol = exp(att - C) for all tiles at once
            ecol = wk.tile([BLK, T], f32, name="ecol", tag="ecol")
            nc.scalar.activation(ecol[:], att_blk[:], Act.Exp,
                                 bias=negC[:, :1])
            # s_col[j] = sum_e oh[e, j] * ecol[e]
            s_ps = acc.tile([BLK, 1], f32, name="s_ps", tag="srl")
            for k in range(T):
                nc.tensor.matmul(out=s_ps[:], lhsT=oh[:, k * BLK:(k + 1) * BLK],
                                 rhs=ecol[:, k:k + 1], start=(k == 0),
                                 stop=(k == T - 1))
            s_sb = wk.tile([BLK, 1], f32, name="s_sb", tag="s_sb")
            nc.vector.tensor_scalar(out=s_sb[:], in0=s_ps[:], scalar1=1e-30,
                                    scalar2=None, op0=Alu.max)
            rinv = wk.tile([BLK, 1], f32, name="rinv", tag="rinv")
            nc.vector.reciprocal(rinv[:], s_sb[:])
            # rl[e] = rinv[dst_lane[e]]  (one-hot matmul, N=1)
            rl_ps = acc.tile([BLK, T], f32, name="rl_ps", tag="srl")
            for k in range(T):
                nc.tensor.matmul(out=rl_ps[:, k:k + 1],
                                 lhsT=ohT[:, k * BLK:(k + 1) * BLK],
                                 rhs=rinv[:], start=True, stop=True)
            # w = ecol * rl  (resident for phases B/C)
            w_b = w_all[:, b * T:(b + 1) * T]
            nc.vector.tensor_tensor(out=w_b, in0=ecol[:], in1=rl_ps[:],
                                    op=Alu.mult)
            # mts = oh * w (bcast along j)
            mts = wk.tile([BLK, EPB], f32, name="mtsA", tag="mts")
            nc.vector.tensor_tensor(
                out=mts[:].rearrange("p (t j) -> p t j", t=T),
                in0=oh[:].rearrange("p (t j) -> p t j", t=T),
                in1=bcast3(w_b, BLK), op=Alu.mult)
            # side[j, feat] = sum_e mts[e, j] * h[e, feat]
            side_ps = acc.tile([BLK, 128], f32, name="side_ps", tag="side")
            for k in range(T):
                nc.tensor.matmul(out=side_ps[:],
                                 lhsT=mts[:, k * BLK:(k + 1) * BLK],
                                 rhs=h_slab[:, k * 128:(k + 1) * 128],
                                 start=(k == 0), stop=(k == T - 1))
            if _STAGE < 6:
                continue
            x1 = wk.tile([BLK, 128], f32, name="x1", tag="x1")
            nc.vector.tensor_tensor(out=x1[:], in0=eblk[:], in1=side_ps[:],
                                    op=Alu.add)
            x2 = wk.tile([BLK, 128], f32, name="x2", tag="x2")
            nc.vector.tensor_tensor(out=x2[:], in0=eblk[:], in1=side_ps[:],
                                    op=Alu.mult)
            ego1_b = ego1_sb[:, b * 64:(b + 1) * 64]
            gemm_norm(x1, x2, 0, 128, 64, ego1_b, eg1sh, 128, b)

        if "B" in _PHASES:
            nc.gpsimd.collective_compute(
                "AllGather", mybir.AluOpType.bypass, replica_groups=rg,
                ins=[eg1sh[:]], outs=[eg1full[:]])

        # ================= phases B (layer 1) and C (layer 2) ============
        for phase, (din, dout, egfull, egsh_next, ego_in, ego_next, ocol) in {
            "B": (64, 32, eg1full, eg2sh, ego1_sb, ego2_sb, 192),
            "C": (32, 16, eg2full, None, ego2_sb, None, 224),
        }.items():
            if phase not in _PHASES:
                continue
            l = 1 if phase == "B" else 2
            for b in range(NBC):
                idx_b = idx_all[:, b * T:(b + 1) * T]
                g_slab = gat.tile([BLK, T * din], f32, name=f"g_slab{l}",
                                  tag=f"g_slab{l}")
                for k in range(T):
                    nc.gpsimd.indirect_dma_start(
                        out=g_slab[:, k * din:(k + 1) * din], out_offset=None,
                        in_=egfull[:],
                        in_offset=IndirectOffsetOnAxis(ap=idx_b[:, k:k + 1],
                                                       axis=0))
                oh = build_oh(dl_all[:, b * T:(b + 1) * T], f"oh{l}")
                mts = wk.tile([BLK, EPB], f32, name=f"mts{l}", tag="mts")
                nc.vector.tensor_tensor(
                    out=mts[:].rearrange("p (t j) -> p t j", t=T),
                    in0=oh[:].rearrange("p (t j) -> p t j", t=T),
                    in1=bcast3(w_all[:, b * T:(b + 1) * T], BLK), op=Alu.mult)
                side_ps = acc.tile([BLK, 128], f32, name=f"sps{l}",
                                   tag="side")[:, :din]
                for k in range(T):
                    nc.tensor.matmul(out=side_ps[:],
                                     lhsT=mts[:, k * BLK:(k + 1) * BLK],
                                     rhs=g_slab[:, k * din:(k + 1) * din],
                                     start=(k == 0), stop=(k == T - 1))
                ego_b = ego_in[:, b * din:(b + 1) * din]
                x1 = wk.tile([BLK, din], f32, name=f"x1{l}", tag="x1")
                nc.vector.tensor_tensor(out=x1[:], in0=ego_b, in1=side_ps[:],
                                        op=Alu.add)
                x2 = wk.tile([BLK, din], f32, name=f"x2{l}", tag="x2")
                nc.vector.tensor_tensor(out=x2[:], in0=ego_b, in1=side_ps[:],
                                        op=Alu.mult)
                if ego_next is not None:
                    ego_o = ego_next[:, b * dout:(b + 1) * dout]
                else:
                    ego_o_t = wk.tile([BLK, dout], f32, name="ego3", tag="ego3")
                    ego_o = ego_o_t[:, :]
                gemm_norm(x1, x2, l, din, dout, ego_o, egsh_next, ocol, b)
            if phase == "B":
                nc.gpsimd.collective_compute(
                    "AllGather", mybir.AluOpType.bypass, replica_groups=rg,
                    ins=[eg2sh[:]], outs=[eg2full[:]])

    nc.compile()
    return nc


# ---------------------------------------------------------------- runner
def run(inputs, cfg, trace=False):
    from concourse.bass_utils import run_bass_kernel_spmd
    ent = np.ascontiguousarray(np.asarray(inputs["ent_embed"], np.float32))
    rel = np.ascontiguousarray(np.asarray(inputs["rel_embed"], np.float32))
    src = np.asarray(inputs["edge_src"]); dst = np.asarray(inputs["edge_dst"])
    typ = np.asarray(inputs["edge_type"])
    prep = _prep(ent, src, dst, typ, cfg)

    key = (cfg.n_nodes, cfg.nbc, cfg.t)
    if key not in _CACHE:
        _CACHE[key] = _build(cfg)
    nc = _CACHE[key]

    in_maps = []
    for c in range(NCORES):
        m = {"ent_slot": prep["ent_slot"],
             "ent_loc": np.ascontiguousarray(
                 prep["ent_slot"][c * cfg.nslot_core:(c + 1) * cfg.nslot_core]),
             "rel": rel}
        for l in range(3):
            for nm in ("W1", "W2"):
                m[f"{nm}_{l}"] = np.ascontiguousarray(
                    np.asarray(inputs[f"{nm}_{l}"], np.float32))
        for nm in ("srcslot", "dstf", "dl_row", "ty_row"):
            m[nm] = np.ascontiguousarray(prep[nm][c])
        in_maps.append(m)

    res = run_bass_kernel_spmd(nc, in_maps, core_ids=list(range(NCORES)),
                               trace=trace)
    out_full = np.zeros((cfg.n_nodes, 240), np.float32)
    co, bo, lo = prep["core_of"], prep["blk_of"], prep["lane_of"]
    for c in range(NCORES):
        o = res.results[c]["out"]
        sel = co == c
        out_full[sel] = o[bo[sel] * BLK + lo[sel]]
    return out_full, res


def kernel(**inputs):
    out, _ = run(inputs, FULL_CFG)
    return out
